# revision 1
# baseline (speedup 1.0000x reference)
"""Trainium2 Bass kernel for DistillLossSimpleMSE (segment_reduce).

Math (per object o, with uniform segments of P points):
    x   = net_out[o*P:(o+1)*P]                [P, D]
    m   = mask_pts[o]                         [M, P] in {0,1}
    e   = nan_to_num(mask_embs[o*M:(o+1)*M])  [M, D]
    sum_sq = sum_m [ sum_p m*||x_p||^2 + cnt_m*||e_m||^2 - 2 e_m . (sum_p m x_p) ]
    out = sum_sq / (D * total_points)

Sharding: object-parallel, 1 object per core (8 objects, 8 cores).

Device kernel per core accumulates in PSUM over all P points (bf16 matmuls,
f32 PSUM accumulate):
    acc[32, 384] = m^T.T @ [x | x*x | 1]
      cols   0:128 -> mx[m, d],  cols 128:256 -> sum_p m x^2,  col 256 -> cnt
Host does the tiny per-mask finale with the embeddings.

Layout trick for full DMA bandwidth: a straight contiguous [128, 4096] tile of
x (16 KB/partition descriptors) has partition p holding rows 32p..32p+31, so
its column-slice [:, c*128:(c+1)*128] is exactly the [point, d] matmul operand
for the stride-32 point class {base + 32p + c}. The mask (contiguous [128,
16384] int32 view) is transposed on-chip through the PE with a stride-32 free
AP so its chunks cover the same point classes.

The rhs buffer holds three contiguous 4096-col bf16 regions [x | x^2 | ones];
the matmul rhs AP gathers one 128-col slice from each region with a regular
two-level access pattern (stride 4096), so x only needs one fast contiguous
convert (DVE 2x mode) and one contiguous square — no strided interleave
copies. The ones region makes out cols 256:384 all equal cnt.

Multi-wait instructions are legalized via bass_rust.generate_event_semaphores
(TRN2 allows only one semaphore wait per compute instruction).
"""

import os

import numpy as np
import ml_dtypes

import bass_rust
import concourse.bass as bass
import concourse.mybir as mybir
import concourse.tile as tile
from concourse.bass_utils import run_bass_kernel_spmd

N_CORES = 8
N_OBJ, P, M, D = 8, 65536, 32, 128

VIEW_P = 128                 # mask flat view partitions
VIEW_F = M * P // VIEW_P     # 16384 view cols; view[r, f] = mask[r//4, (r%4)*16384 + f]
BLK = 4096                   # view cols per block (= points per x-tile)
NBLK = VIEW_F // BLK         # 4 mask blocks
NCLS = BLK // 128            # 32 stride-32 point classes per block
NT = 16                      # x tiles of [128, 4096]
OUTC = 2 * D                 # 256 output cols: [mx | m@x^2]; cnt via convert accum
NXB = 3                      # x-tile landing buffers (f32)
NRHS = 5                     # rhs buffers [x | x^2] bf16

F32 = mybir.dt.float32
BF16 = mybir.dt.bfloat16
I32 = mybir.dt.int32

LAST = None      # BassKernelResults of the most recent run (for test harness)
_NC_CACHE = {}


def _build_nc():
    nc = bass.Bass()
    x = nc.dram_tensor("x", [P, D], F32, kind="ExternalInput")
    mask = nc.dram_tensor("mask", [VIEW_P, VIEW_F], I32, kind="ExternalInput")
    out = nc.dram_tensor("out", [M, OUTC], F32, kind="ExternalOutput")
    # per-(mask,quarter)-row, per-piece partial point counts (see host finale)
    cnts = nc.dram_tensor("cnts", [VIEW_P, NBLK * 4], F32, kind="ExternalOutput")

    # x tile view: [32 tiles, 128 partitions, 16*128 contiguous]
    xt = x[:, :].rearrange("(j p s) d -> j p (s d)", p=128, s=BLK // 128)

    with tile.TileContext(nc) as tc:
        with (
            tc.tile_pool(name="singles", bufs=1) as singles,
            tc.tile_pool(name="psingles", bufs=1, space="PSUM") as psingles,
        ):
            # Persistent tiles only: pool-reallocated tiles go through Tile's
            # release machinery whose extra waits collide with the PE 1-wait
            # codegen limit more often.
            ident_const = nc.inline_tensor(
                np.eye(128, dtype=np.float32).astype(ml_dtypes.bfloat16),
                name="identc",
            )
            ident = singles.tile([128, 128], BF16, tag="ident")
            nc.sync.dma_start(out=ident, in_=ident_const[:, :])

            # rhs buffers: per-chunk contiguous [x | x^2] (one-segment matmul
            # rhs APs; multi-segment APs split into sub-matmuls on HW).
            rhs_bufs = [
                singles.tile(
                    [128, NCLS, 2 * D], BF16, name=f"rhsbuf{j}", tag=f"rhsbuf{j}"
                )
                for j in range(NRHS)
            ]
            cnt_sb = singles.tile([VIEW_P, NBLK * 4], F32, tag="cnt_sb")

            xb_bufs = [
                singles.tile([128, BLK], F32, name=f"xb{j}", tag=f"xb{j}")
                for j in range(NXB)
            ]
            MIP = BLK // 4
            mi_bufs = [
                singles.tile([VIEW_P, MIP], I32, name=f"mi{j}", tag=f"mi{j}")
                for j in range(3)
            ]
            mf_bufs = [
                singles.tile([VIEW_P, BLK], BF16, name=f"mf{j}", tag=f"mf{j}")
                for j in range(2)
            ]
            # One mt per block (written once, never reused) so the whole mask
            # pipeline can run ahead of the matmul stream.
            mt_bufs = [
                singles.tile([VIEW_P, BLK], BF16, name=f"mt{j}", tag=f"mt{j}")
                for j in range(NBLK)
            ]
            ps4_bufs = [
                psingles.tile([128, 8, 128], BF16, name=f"ps4{j}", tag=f"ps4{j}")
                for j in range(2)
            ]
            acc = psingles.tile([M, OUTC], F32, tag="acc")

            n_mm = NBLK * 4 * NCLS

            def mask_chain(b):
                mf = mf_bufs[b % 2]
                # Piecewise DMA + convert through small rotating int32 pieces.
                # DVE, not GpSimd: the GpSimd CAST is ~4x slower and its
                # SBUF port contention starves concurrent DVE ops.
                for piece in range(4):
                    mi = mi_bufs[(b * 4 + piece) % 3]
                    lo, hi = piece * MIP, (piece + 1) * MIP
                    nc.sync.dma_start(
                        out=mi, in_=mask[:, b * BLK + lo:b * BLK + hi]
                    )
                    # convert + free-dim count in one DVE op
                    nc.vector.scalar_tensor_tensor(
                        out=mf[:, lo:hi],
                        in0=mi, scalar=0.0, in1=mi,
                        op0=mybir.AluOpType.add,
                        op1=mybir.AluOpType.bypass,
                        accum_out=cnt_sb[:, b * 4 + piece:b * 4 + piece + 1],
                    )

            def transposes(b, h0, h1):
                # Transpose the mask block through the PE with stride-32 free
                # APs: transpose c yields, for every quarter q, the lhsT mask
                # columns of point class {q*16384 + b*4096 + 32p + c}.
                mf = mf_bufs[b % 2]
                mt = mt_bufs[b]
                # f' = 32p + c: class c picks stride-32 free elems
                mfv = mf.rearrange("r (p c) -> r c p", c=NCLS)
                for h in range(h0, h1):
                    ps4 = ps4_bufs[h % 2]
                    for tt in range(8):
                        c = h * 8 + tt
                        nc.tensor.transpose(ps4[:, tt, :], mfv[:, c, :], ident)
                    nc.scalar.copy(
                        mt[:, h * 1024:(h + 1) * 1024],
                        ps4.rearrange("p t d -> p (t d)"),
                    )

            k = 0
            jx = 0
            mask_chain(0)
            transposes(0, 0, NCLS // 8)
            for b in range(NBLK):
                mtv = mt_bufs[b].rearrange("p (c m q) -> p c q m", c=NCLS, m=M, q=4)
                for q in range(4):
                    j = q * NBLK + b       # x tile covering this block+quarter
                    xb = xb_bufs[jx % NXB]
                    rhs = rhs_bufs[jx % NRHS]
                    jx += 1
                    nc.sync.dma_start(out=xb, in_=xt[j, :, :])
                    xbv = xb.rearrange("p (s d) -> p s d", s=NCLS)
                    # strided f32->bf16 convert, alternating ACT / DVE
                    # (tensor_scalar has uops for the fast DVE modes)
                    if (b + q) % 2 == 0:
                        nc.scalar.copy(rhs[:, :, 0:D], xbv)
                    else:
                        nc.vector.tensor_scalar_mul(rhs[:, :, 0:D], xbv, 1.0)
                    # strided bf16 square on DVE (1x, but PE matmul gets a
                    # single contiguous 256-col segment)
                    nc.vector.tensor_mul(
                        rhs[:, :, D:2 * D], rhs[:, :, 0:D], rhs[:, :, 0:D]
                    )
                    for c in range(NCLS):
                        nc.tensor.matmul(
                            acc[:, :],
                            lhsT=mtv[:, c, q, :],
                            rhs=rhs[:, c, :],
                            start=(k == 0),
                            stop=(k == n_mm - 1),
                        )
                        k += 1
                    # Software pipeline: next block's mask work interleaves
                    # with this block's matmul groups so the PE never drains
                    # at a block boundary.
                    if b + 1 < NBLK:
                        if q == 0:
                            mask_chain(b + 1)
                        if q >= 2:
                            h0 = (q - 2) * 2
                            transposes(b + 1, h0, h0 + 2)

            outs = singles.tile([M, OUTC], F32, tag="outs")
            nc.vector.tensor_copy(outs, acc)
            nc.sync.dma_start(out=out[:, :], in_=outs)
            nc.sync.dma_start(out=cnts[:, :], in_=cnt_sb)
    # Split multi-wait instructions into EventSemaphore + instruction to
    # satisfy the TRN2 1-wait-per-instruction codegen limit.
    bass_rust.generate_event_semaphores(nc)
    return nc


def _get_nc():
    if "nc" not in _NC_CACHE:
        _NC_CACHE["nc"] = _build_nc()
    return _NC_CACHE["nc"]


def kernel(net_out, pt_offset, mask_embs, mask_pts, logit_scale):
    global LAST
    net_out = np.ascontiguousarray(np.asarray(net_out, dtype=np.float32))
    mask_pts = np.ascontiguousarray(np.asarray(mask_pts, dtype=np.int32))
    mask_embs = np.asarray(mask_embs, dtype=np.float32)

    nc = _get_nc()
    in_maps = [
        {
            "x": net_out[o * P:(o + 1) * P],
            "mask": mask_pts[o].reshape(VIEW_P, VIEW_F),
        }
        for o in range(N_CORES)
    ]
    trace = os.environ.get("KBENCH_TRACE", "0") == "1"
    res = run_bass_kernel_spmd(nc, in_maps, list(range(N_CORES)), trace=trace)
    LAST = res

    accs = np.stack([np.asarray(res.results[o]["out"]) for o in range(N_CORES)])
    mx = accs[:, :, 0:D].astype(np.float64)        # [8, 32, 128]
    sx2 = accs[:, :, D:2 * D].astype(np.float64)   # [8, 32, 128]
    # cnt[m] = sum over quarters q and pieces of the per-row partials
    cnts = np.stack([np.asarray(res.results[o]["cnts"]) for o in range(N_CORES)])
    cnt = cnts.sum(-1).reshape(N_CORES, M, 4).sum(-1)  # [8, 32]

    emb = np.nan_to_num(
        mask_embs.reshape(N_OBJ, M, D).astype(np.float64),
        nan=0.0, posinf=0.0, neginf=0.0,
    )
    t1 = sx2.sum(-1)
    t2 = cnt * (emb * emb).sum(-1)
    t3 = 2.0 * (emb * mx).sum(-1)
    sum_sq = (t1 + t2 - t3).sum()
    total = cnt.sum()
    val = sum_sq / (D * total) if total > 0 else 0.0
    return np.float32(val)



# revision 2
# speedup vs baseline: 1.0669x; 1.0669x over previous
"""Trainium2 Bass kernel for DistillLossSimpleMSE (segment_reduce).

Math (per object o, with uniform segments of P points):
    x   = net_out[o*P:(o+1)*P]                [P, D]
    m   = mask_pts[o]                         [M, P] in {0,1}
    e   = nan_to_num(mask_embs[o*M:(o+1)*M])  [M, D]
    sum_sq = sum_m [ sum_p m*||x_p||^2 + cnt_m*||e_m||^2 - 2 e_m . (sum_p m x_p) ]
    out = sum_sq / (D * total_points)

Sharding: object-parallel, 1 object per core (8 objects, 8 cores).

Device kernel per core accumulates in PSUM over all P points:
    acc[32, 256] = m^T.T @ [x | x*x]
      cols 0:128 -> mx[m, d],  cols 128:256 -> sum_p m x^2 per d
Host does the tiny per-mask finale with the embeddings; per-mask point
counts are a cheap host-side mask_pts.sum().

All input DMAs are SWDGE (gpsimd ring) casting DMAs, which run at full
HBM rate (measured ~equal to plain HWDGE):
  - x tiles: f32 -> bf16, landing contiguously in region 0 of an
    [128, 2, 32, 128] "xcomb" tile. DVE squares region 0 into region 1
    (contiguous bf16, 2x mode). The class matmul reads a 2-segment rhs
    AP [x_c | x^2_c] (stride 4096) which the PE executes as sub-matmuls
    under a single LDWEIGHTS at the same issue rate as contiguous rhs.
  - mask blocks: i32 -> bf16 straight into mf (no int staging, no DVE
    convert, no on-device counts).

The mask is transposed on-chip through the PE with stride-32 free APs
(classes of stride-32 points match the x-tile partition layout), then
copied PSUM->SBUF on the ACT engine.

x tiles are loaded in two 2048-point halves so the final tile's
square+matmul chain after the last DMA byte is short, and squares are
per-half so matmuls for classes 0:16 start after half 1.

Multi-wait instructions are legalized via bass_rust.generate_event_semaphores
(TRN2 allows only one semaphore wait per compute instruction).
"""

import os

import numpy as np
import ml_dtypes

import bass_rust
import concourse.bass as bass
import concourse.mybir as mybir
import concourse.tile as tile
from concourse.bass_utils import run_bass_kernel_spmd

N_CORES = 8
N_OBJ, P, M, D = 8, 65536, 32, 128

VIEW_P = 128                 # mask flat view partitions
VIEW_F = M * P // VIEW_P     # 16384 view cols; view[r, f] = mask[r//4, (r%4)*16384 + f]
BLK = 4096                   # view cols per block (= points per x-tile)
NBLK = VIEW_F // BLK         # 4 mask blocks
NCLS = BLK // 128            # 32 stride-32 point classes per block
NT = 16                      # x tiles of [128, 4096]
OUTC = 2 * D                 # 256 output cols: [mx | m@x^2]
NXC = 8                      # xcomb landing buffers (bf16, [x | x^2])
NMF = 3                      # mf mask landing buffers

F32 = mybir.dt.float32
BF16 = mybir.dt.bfloat16
I32 = mybir.dt.int32

LAST = None      # BassKernelResults of the most recent run (for test harness)
_NC_CACHE = {}


def _build_nc():
    nc = bass.Bass()
    x = nc.dram_tensor("x", [P, D], F32, kind="ExternalInput")
    mask = nc.dram_tensor("mask", [VIEW_P, VIEW_F], I32, kind="ExternalInput")
    out = nc.dram_tensor("out", [M, OUTC], F32, kind="ExternalOutput")

    # x tile view: [16 tiles, 128 partitions, 32*128 contiguous]
    xt = x[:, :].rearrange("(j p s) d -> j p (s d)", p=128, s=BLK // 128)

    with tile.TileContext(nc) as tc:
        with (
            tc.tile_pool(name="singles", bufs=1) as singles,
            tc.tile_pool(name="psingles", bufs=1, space="PSUM") as psingles,
        ):
            # Persistent tiles only: pool-reallocated tiles go through Tile's
            # release machinery whose extra waits collide with the PE 1-wait
            # codegen limit more often.
            ident_const = nc.inline_tensor(
                np.eye(128, dtype=np.float32).astype(ml_dtypes.bfloat16),
                name="identc",
            )
            ident = singles.tile([128, 128], BF16, tag="ident")
            nc.sync.dma_start(out=ident, in_=ident_const[:, :])

            # [x | x^2] combined tiles: region r=0 holds the cast x tile
            # (contiguous DMA dst), r=1 the squares; the matmul rhs AP
            # [:, :, c, :] gathers class c from both regions (2 segments).
            xc_bufs = [
                singles.tile([128, 2, NCLS, D], BF16, name=f"xc{j}", tag=f"xc{j}")
                for j in range(NXC)
            ]
            mf_bufs = [
                singles.tile([VIEW_P, BLK], BF16, name=f"mf{j}", tag=f"mf{j}")
                for j in range(NMF)
            ]
            # One mt per block (written once, never reused) so the whole mask
            # pipeline can run ahead of the matmul stream.
            mt_bufs = [
                singles.tile([VIEW_P, BLK], BF16, name=f"mt{j}", tag=f"mt{j}")
                for j in range(NBLK)
            ]
            ps4_bufs = [
                psingles.tile([128, 8, 128], BF16, name=f"ps4{j}", tag=f"ps4{j}")
                for j in range(2)
            ]
            acc = psingles.tile([M, OUTC], F32, tag="acc")

            n_mm = NBLK * 4 * NCLS

            def mask_dma(b):
                # casting DMA: i32 HBM -> bf16 SBUF, one 2 MiB op per block
                nc.gpsimd.dma_start(
                    out=mf_bufs[b % NMF], in_=mask[:, b * BLK:(b + 1) * BLK]
                )

            def transposes(b, h0, h1):
                # Transpose the mask block through the PE with stride-32 free
                # APs: transpose c yields, for every quarter q, the lhsT mask
                # columns of point class {q*16384 + b*4096 + 32p + c}.
                mf = mf_bufs[b % NMF]
                mt = mt_bufs[b]
                # f' = 32p + c: class c picks stride-32 free elems
                mfv = mf.rearrange("r (p c) -> r c p", c=NCLS)
                for h in range(h0, h1):
                    ps4 = ps4_bufs[h % 2]
                    for tt in range(8):
                        c = h * 8 + tt
                        nc.tensor.transpose(ps4[:, tt, :], mfv[:, c, :], ident)
                    nc.scalar.copy(
                        mt[:, h * 1024:(h + 1) * 1024],
                        ps4.rearrange("p t d -> p (t d)"),
                    )

            k = 0
            jx = 0
            mask_dma(0)
            transposes(0, 0, NCLS // 8)
            for b in range(NBLK):
                mtv = mt_bufs[b].rearrange("p (c m q) -> p c q m", c=NCLS, m=M, q=4)
                for q in range(4):
                    j = q * NBLK + b   # x tile covering this block+quarter
                    xc = xc_bufs[jx % NXC]
                    jx += 1
                    xcf = xc.rearrange("p r c d -> p (r c d)")
                    # two half-tile casting DMAs + per-half contiguous squares
                    # so matmuls start after half 1 and the tail chain after
                    # the very last DMA byte is only half a tile deep
                    HB = BLK // 2
                    nc.gpsimd.dma_start(
                        out=xcf[:, 0:HB], in_=xt[j, :, 0:HB]
                    )
                    nc.gpsimd.dma_start(
                        out=xcf[:, HB:BLK], in_=xt[j, :, HB:BLK]
                    )
                    nc.vector.tensor_mul(
                        xcf[:, BLK:BLK + HB], xcf[:, 0:HB], xcf[:, 0:HB]
                    )
                    nc.vector.tensor_mul(
                        xcf[:, BLK + HB:2 * BLK], xcf[:, HB:BLK], xcf[:, HB:BLK]
                    )
                    for c in range(NCLS):
                        nc.tensor.matmul(
                            acc[:, :],
                            lhsT=mtv[:, c, q, :],
                            rhs=xc[:, :, c, :],
                            start=(k == 0),
                            stop=(k == n_mm - 1),
                        )
                        k += 1
                    # Software pipeline: next block's mask work interleaves
                    # with this block's matmul groups so the PE never drains
                    # at a block boundary.
                    if b + 1 < NBLK:
                        if q == 0:
                            mask_dma(b + 1)
                        if q >= 2:
                            h0 = (q - 2) * 2
                            transposes(b + 1, h0, h0 + 2)

            outs = singles.tile([M, OUTC], F32, tag="outs")
            nc.vector.tensor_copy(outs, acc)
            nc.sync.dma_start(out=out[:, :], in_=outs)
    # Split multi-wait instructions into EventSemaphore + instruction to
    # satisfy the TRN2 1-wait-per-instruction codegen limit.
    bass_rust.generate_event_semaphores(nc)
    return nc


def _get_nc():
    if "nc" not in _NC_CACHE:
        _NC_CACHE["nc"] = _build_nc()
    return _NC_CACHE["nc"]


def kernel(net_out, pt_offset, mask_embs, mask_pts, logit_scale):
    global LAST
    net_out = np.ascontiguousarray(np.asarray(net_out, dtype=np.float32))
    mask_pts = np.ascontiguousarray(np.asarray(mask_pts, dtype=np.int32))
    mask_embs = np.asarray(mask_embs, dtype=np.float32)

    nc = _get_nc()
    in_maps = [
        {
            "x": net_out[o * P:(o + 1) * P],
            "mask": mask_pts[o].reshape(VIEW_P, VIEW_F),
        }
        for o in range(N_CORES)
    ]
    trace = os.environ.get("KBENCH_TRACE", "0") == "1"
    res = run_bass_kernel_spmd(nc, in_maps, list(range(N_CORES)), trace=trace)
    LAST = res

    accs = np.stack([np.asarray(res.results[o]["out"]) for o in range(N_CORES)])
    mx = accs[:, :, 0:D].astype(np.float64)        # [8, 32, 128]
    sx2 = accs[:, :, D:2 * D].astype(np.float64)   # [8, 32, 128]
    cnt = mask_pts.sum(axis=2, dtype=np.int64)     # [8, 32] host-side counts

    emb = np.nan_to_num(
        mask_embs.reshape(N_OBJ, M, D).astype(np.float64),
        nan=0.0, posinf=0.0, neginf=0.0,
    )
    t1 = sx2.sum(-1)
    t2 = cnt * (emb * emb).sum(-1)
    t3 = 2.0 * (emb * mx).sum(-1)
    sum_sq = (t1 + t2 - t3).sum()
    total = cnt.sum()
    val = sum_sq / (D * total) if total > 0 else 0.0
    return np.float32(val)
